# revision 49
# baseline (speedup 1.0000x reference)
"""Trainium2 Bass kernel for nn_MiniGRUConv2d4 (MinGRU 4-direction conv scan).

Problem (B=4, Cin=64, Cout4=256, H=W=256):
    u_c  = conv3x3(xs, w_c) + bn_c          for c in {z, h, s}   (Cout=256)
    z    = sigmoid(u_z); hh = u_h; s = sigmoid(u_s)
    split 256 channels into 4 groups of 64; group g scans
      g=0: over H fwd, g=1: over H rev, g=2: over W fwd, g=3: over W rev
      h_i = z_i*hh_i + (1-z_i)*h_{i-1}
    out  = sum_g s_g * h_g                  (B, 64, H, W)

Sharding (8 cores): core = (batch b, orientation o).
  o=0: natural image, conv channels 128..255 (groups 2,3: W-fwd / W-rev)
  o=1: transposed image (host transposes), channels 0..127 (groups 0,1:
       H-scan becomes W-scan in the transposed frame).
Every core runs the identical program; group A (partitions 0:64) scans
forward along W, group B (partitions 64:128) scans backward (negative-
stride APs feeding the hardware scan instruction). The conv is 6 K=128
fp16 matmuls per 2-row x 256-col tile (3x3 taps: dy0/dy1 pairs packed
into the 128-partition contraction via a row-shifted second copy of the
input; dy2 rides in the upper 64 partitions with zero top-half weights).

Engine budget per core (measured): PE ~450us of matmul (the fp16 FLOP
floor is ~410us; fp8 was measured numerically and its conv error alone
breaks the 2e-2 gate, so fp16 it is), DVE ~430us (hw scan is ~2.24
ns/elem, serial in the free dim), ACT ~257us, DMA ~220us. The scan runs
UNSPLIT (channel-major) to keep DMA traffic low -- an earlier variant
row-split the scan via SBUF<->SBUF remaps to halve DVE time, but the
remap traffic saturated the 16 DMA engines (~445us busy) and starved
the PE's input loads. Instead the scan is emitted as four quarter-band
chunks interleaved between the NEXT band's b-computations, so the
in-order DVE queue never blocks the PSUM drain for more than ~2.3us.
"""

import sys
import types

import numpy as np

import concourse.bass as bass
import concourse.mybir as mybir
import concourse.tile as tile

F32 = mybir.dt.float32
AF = mybir.ActivationFunctionType
OP = mybir.AluOpType

_R = 8  # band height (output rows per band)


# ---------------------------------------------------------------------------
# Workaround: the pinned walrus rejects instructions carrying more than a
# couple of sem waits ("Too many sync wait commands", CoreV3GenImpl
# setupSyncWait). Hoist excess waits onto same-engine NOPs inserted right
# before the offending instruction.
_MAX_WAITS = 1


def _split_excess_waits(nc, max_waits=_MAX_WAITS):
    import bass_rust

    n_split = 0
    for f in nc.m.functions:
        for blk in f.blocks:
            out = []
            for inst in blk.instructions:
                si = inst.sync_info
                if si is not None and len(si.on_wait) > max_waits:
                    waits = list(si.on_wait)
                    extra, keep = waits[:-max_waits], waits[-max_waits:]
                    for i0 in range(0, len(extra), max_waits):
                        nop = mybir.InstNoOp(
                            name=f"{inst.name}_xw{i0}", ins=[], outs=[]
                        )
                        nop.engine = inst.engine
                        nop.sync_info = bass_rust.SyncInfo(
                            on_wait=extra[i0 : i0 + max_waits], on_update=[]
                        )
                        nc.register_instruction(nop)
                        out.append(nop)
                        n_split += 1
                    inst.sync_info = bass_rust.SyncInfo(
                        on_wait=keep, on_update=list(si.on_update)
                    )
                out.append(inst)
            blk.instructions = out
    return n_split


def _ensure_axon_hooks_importable():
    # bass_utils imports antenv.axon_hooks when tracing is requested; the
    # container's antenv stub lacks it. Provide a no-op registry so the
    # import never crashes (tracing then just degrades gracefully).
    try:
        import antenv.axon_hooks  # noqa: F401
    except Exception:
        try:
            import antenv

            mod = types.ModuleType("antenv.axon_hooks")
            mod._hook = None
            mod.set_axon_ntff_profile_hook = lambda h: setattr(mod, "_hook", h)
            mod.get_axon_ntff_profile_hook = lambda: mod._hook
            sys.modules["antenv.axon_hooks"] = mod
            antenv.axon_hooks = mod
        except Exception:
            pass


# ---------------------------------------------------------------------------
# Device program

# Conv operands: fp16 runs the PE at full rate (1 cyc/row) with a 10-bit
# mantissa -- conv error ~5e-4. fp8 (e4m3, 2x rate w/ DoubleRow) was
# measured at 4.6e-2 final rel err: rejected.
CONV_DT = mybir.dt.float16
CHAIN_DT = mybir.dt.float16  # z/s/a/b/h/p tiles + output (host upcasts)
# Row-split group A's scan via one SBUF->SBUF remap (halves A's serial scan
# length; B stays unsplit). Measured SLOWER (476-485us vs 453us): the remap
# saves ~66us of DVE but its SBUF<->SBUF traffic inflates PE throughput and
# adds input-load micro-stalls, and PE is the binding engine. Keep False.
SPLIT_A = False
_PADC = 5  # extra pad columns: Wp = W + 5 so every x DMA is one flat
#            descriptor per partition (row stride == Wp, junk cols are
#            zero-padded and only ever hit zero weights)


def build_nc(H, W, with_init_fixup=True):
    """One-core program; all 8 cores run it SPMD with different inputs."""
    R = _R
    RR = R + 1  # input rows resident per band (dy0/dy1 buffer)
    Wp = W + _PADC
    assert H % R == 0 and W % 2 == 0
    nbands = H // R
    cdt = CONV_DT
    wdt = CHAIN_DT
    half = R * W // 2

    nc = bass.Bass("TRN2", target_bir_lowering=False, debug=False)
    # flat padded image: row y, col k at offset y*Wp + k; rows 0..H+2
    # (top pad 1, bottom pad 2), cols: k=0 left pad, k in 1..W image,
    # k in W+1..Wp-1 right pad/junk (zeroed by host).
    xp = nc.dram_tensor("xp", [64, (H + 3) * Wp], cdt, kind="ExternalInput").ap()
    wts = nc.dram_tensor("wts", [128, 15, 128], cdt, kind="ExternalInput").ap()
    consts = nc.dram_tensor("consts", [128, 4], F32, kind="ExternalInput").ap()
    out = nc.dram_tensor("out", [128, H * W], wdt, kind="ExternalOutput").ap()

    with tile.TileContext(nc) as tc:
        with (
            tc.tile_pool(name="const", bufs=1) as cpool,
            tc.tile_pool(name="xin", bufs=4) as xpool,
            tc.tile_pool(name="work", bufs=4) as wpool,
            tc.tile_pool(name="psum", bufs=2, space="PSUM") as ppool,
        ):
            wts_sb = cpool.tile([128, 15, 128], cdt)
            nc.sync.dma_start(wts_sb[:], wts)
            cst = cpool.tile([128, 4], F32)
            nc.sync.dma_start(cst[:], consts)
            bias = [cst[:, c : c + 1] for c in range(3)]  # z, h, s
            init = cst[:, 3:4]

            def scan_chunk(k, a_p, b_p, h_p):
                # whole-group scan chunks: k=0 -> group A forward, k=1 ->
                # group B backward, each one instruction over the full band
                # (fewer DVE instruction overheads than quarter-band chunks;
                # the scan is the pacing engine's biggest block).
                if k == 0:
                    nc.vector.tensor_tensor_scan(
                        h_p[0:64, :], a_p[0:64, :], b_p[0:64, :],
                        0.0, op0=OP.mult, op1=OP.add,
                    )
                else:
                    nc.vector.tensor_tensor_scan(
                        h_p[64:128, ::-1], a_p[64:128, ::-1],
                        b_p[64:128, ::-1], 0.0,
                        op0=OP.mult, op1=OP.add,
                    )

            # prev-band state for the software-pipelined back-end
            prev = None  # (a_b, b_b, s_b, h_b, p_b, y0, absA, hsA)
            pending_out = None  # (p_b, y0) store deferred past newer loads

            def back_end_slot(k, pv):
                # DVE/Sync work for the previous band, spread across this
                # band's unit slots. SPLIT_A: k=0 -> group A split scan,
                # k=1,2 -> group B chunks (k=2 also fires the A remap-back
                # triggers, whose scan-A wait is satisfied by then so the
                # in-order Sync stream never stalls), k=3 -> idle.
                a_p, b_p, s_p, h_p, p_p, y0_p, absA_p, hsA_p = pv[:8]
                if SPLIT_A:
                    if k == 0:
                        nc.vector.tensor_tensor_scan(
                            hsA_p[:, :], absA_p[:, 0:half],
                            absA_p[:, half : 2 * half], 0.0,
                            op0=OP.mult, op1=OP.add,
                        )
                    elif k in (1, 2):
                        scan_chunk(k + 1, a_p, b_p, h_p)
                        if k == 2:
                            Rh = R // 2
                            h3 = h_p.rearrange("p (r w) -> p r w", w=W)
                            hsA3 = hsA_p.rearrange("p (r w) -> p r w", w=W)
                            nc.sync.dma_start(h3[0:64, 0:Rh, :], hsA3[0:64])
                            nc.sync.dma_start(
                                h3[0:64, Rh:R, :], hsA3[64:128]
                            )
                else:
                    lst = pv[8]
                    kk = lst[k] if k < len(lst) else None
                    if kk is not None:
                        scan_chunk(kk, a_p, b_p, h_p)

            for band in range(nbands + 1):
                if band < nbands:
                    y0 = band * R
                    # x2: dy0 rows at partitions 0:64, dy1 rows (one row
                    # down) at 64:128. One flat descriptor per partition.
                    x2f = xpool.tile([128, RR * Wp], cdt)
                    x3f = xpool.tile([128, R * Wp], cdt)
                    nc.sync.dma_start(
                        x2f[0:64], xp[:, y0 * Wp : (y0 + RR) * Wp]
                    )
                    nc.sync.dma_start(
                        x2f[64:128],
                        xp[:, (y0 + 1) * Wp : (y0 + 1 + RR) * Wp],
                    )
                    # x3: dy2 rows; lower = col+0, upper = col+1 (flat
                    # offset by one element; the tail reads the next
                    # padded row's col 0, real zero-padded memory).
                    nc.sync.dma_start(
                        x3f[0:64], xp[:, (y0 + 2) * Wp : (y0 + 2 + R) * Wp]
                    )
                    nc.sync.dma_start(
                        x3f[64:128],
                        xp[:, (y0 + 2) * Wp + 1 : (y0 + 2 + R) * Wp + 1],
                    )
                    x2 = x2f.rearrange("p (r w) -> p r w", w=Wp)
                    x3 = x3f.rearrange("p (r w) -> p r w", w=Wp)
                if pending_out is not None:
                    p_o, y0_o = pending_out
                    nc.sync.dma_start(
                        out[:, y0_o * W : (y0_o + R) * W], p_o[:]
                    )
                    pending_out = None

                if band < nbands:
                    z_b = wpool.tile([128, R * W], wdt)
                    s_b = wpool.tile([128, R * W], wdt)
                    ab = wpool.tile([128, 2 * R * W], wdt)
                    a_b = ab[:, 0 : R * W]
                    b_b = ab[:, R * W : 2 * R * W]
                    h_b = wpool.tile([128, R * W], wdt)
                    p_b = wpool.tile([128, R * W], wdt)
                    if SPLIT_A:
                        # split-layout [a|b] for group A: partition =
                        # (row-half, ch), free = [a|b] x (R/2 rows x W)
                        absA = wpool.tile([128, 2 * half], wdt)
                        hsA = wpool.tile([128, half], wdt)
                    else:
                        absA = hsA = None

                    for j0 in range(0, R, 2):  # unit: 2 output rows, N=512
                        us = []
                        for c in range(3):  # z, h, s convs
                            u = ppool.tile(
                                [128, 2 * W], F32, name=f"u{c}", tag=f"u{c}",
                                bufs=(4 if c == 1 else 2),
                            )
                            for dx in range(3):  # dy0+dy1 pairs
                                nc.tensor.matmul(
                                    u[:],
                                    wts_sb[:, 3 * c + dx, :],
                                    x2[:, j0 : j0 + 2, dx : dx + W],
                                    start=(dx == 0),
                                    stop=False,
                                )
                            # dy2 (dx0, dx1) pair
                            nc.tensor.matmul(
                                u[:],
                                wts_sb[:, 9 + c, :],
                                x3[:, j0 : j0 + 2, 0:W],
                                start=False,
                                stop=False,
                            )
                            # dy2 dx2 (upper weights zero -> K=128 uniform)
                            nc.tensor.matmul(
                                u[:],
                                wts_sb[:, 12 + c, :],
                                x3[:, j0 : j0 + 2, 2 : 2 + W],
                                start=False,
                                stop=True,
                            )
                            us.append(u)
                        sl = slice(j0 * W, (j0 + 2) * W)
                        nc.scalar.activation(
                            z_b[:, sl], us[0][:], AF.Sigmoid, bias=bias[0]
                        )
                        nc.scalar.activation(
                            s_b[:, sl], us[2][:], AF.Sigmoid, bias=bias[2]
                        )
                        # b = (u_h + bias_h) * z; reads u_h straight from
                        # PSUM (psum ports are separate from the saturated
                        # SBUF ports)
                        nc.vector.scalar_tensor_tensor(
                            b_b[:, sl], us[1][:], bias[1], z_b[:, sl],
                            op0=OP.add, op1=OP.mult,
                        )
                        # interleave one quarter of the previous band's
                        # back-end behind each unit's b
                        if prev is not None:
                            back_end_slot(j0 // 2, prev)
                else:
                    if prev is not None:
                        for k in range(4):
                            back_end_slot(k, prev)

                if prev is not None:
                    a_p, b_p, s_p, h_p, p_p, y0_p = prev[:6]
                    # (p on GpSimd measured +50us: Q7 elementwise is slow
                    # and its SBUF port locking taxes DVE. Keep on DVE.)
                    nc.vector.tensor_mul(p_p[:], s_p[:], h_p[:])
                    # the store trigger waits on p; emitting it here would
                    # gate the NEXT band's input loads on the in-order Sync
                    # stream. Defer it to the top of the next iteration
                    # (after those loads are already in flight).
                    pending_out = (p_p, y0_p)
                    prev = None

                if band < nbands:
                    # a = 1 - z (on ACT: Identity(-z + 1); DVE is scarce)
                    nc.scalar.activation(
                        a_b[:], z_b[:], AF.Identity, bias=1.0, scale=-1.0
                    )
                    a3 = a_b.rearrange("p (r w) -> p r w", w=W)
                    b3 = b_b.rearrange("p (r w) -> p r w", w=W)
                    # fold the (normally zero) scan init into b at each row
                    # edge, then zero `a` there so the flat scan restarts
                    # per row.
                    if with_init_fixup:
                        nc.vector.scalar_tensor_tensor(
                            b3[0:64, :, 0], a3[0:64, :, 0], init[0:64],
                            b3[0:64, :, 0], op0=OP.mult, op1=OP.add,
                        )
                        nc.vector.scalar_tensor_tensor(
                            b3[64:128, :, W - 1], a3[64:128, :, W - 1],
                            init[64:128], b3[64:128, :, W - 1],
                            op0=OP.mult, op1=OP.add,
                        )
                    nc.scalar.activation(
                        a3[0:64, :, 0], a3[0:64, :, 0], AF.Copy,
                        bias=0.0, scale=0.0,
                    )
                    nc.scalar.activation(
                        a3[64:128, :, W - 1], a3[64:128, :, W - 1],
                        AF.Copy, bias=0.0, scale=0.0,
                    )
                    if SPLIT_A:
                        # remap group A's a|b into the row-split layout:
                        # rows 0:R/2 -> partitions 0:64, rows R/2:R -> 64:128
                        Rh = R // 2
                        ab4 = ab.rearrange("p (pl r w) -> p pl r w", pl=2, w=W)
                        absA4 = absA.rearrange(
                            "p (pl r w) -> p pl r w", pl=2, w=W
                        )
                        nc.sync.dma_start(absA4[0:64], ab4[0:64, :, 0:Rh, :])
                        nc.sync.dma_start(
                            absA4[64:128], ab4[0:64, :, Rh:R, :]
                        )
                    prev = (
                        a_b, b_b, s_b, h_b, p_b, y0, absA, hsA,
                        # A after unit 0's b, B after unit 2's b: spreads the
                        # two 4.6us scan blocks so psum drains stay in slack
                        [0, None, None, 1],
                    )
            if pending_out is not None:
                p_o, y0_o = pending_out
                nc.sync.dma_start(out[:, y0_o * W : (y0_o + R) * W], p_o[:])
    _split_excess_waits(nc)
    return nc


# ---------------------------------------------------------------------------
# Host side

_NC_CACHE = {}


def _get_nc(H, W, with_init_fixup=True):
    key = (H, W, with_init_fixup)
    if key not in _NC_CACHE:
        _NC_CACHE[key] = build_nc(H, W, with_init_fixup)
    return _NC_CACHE[key]


def make_in_maps(inputs, H, W):
    """Build the 8 per-core input dicts from the full problem inputs."""
    xs = np.ascontiguousarray(np.asarray(inputs["xs"], dtype=np.float32))
    B = xs.shape[0]
    Wp = W + _PADC
    Ws, Bs = {}, {}
    for tag in ("z", "h", "s"):
        w = np.asarray(inputs["w_" + tag], dtype=np.float32)
        g = np.asarray(inputs["g_" + tag], dtype=np.float32)
        be = np.asarray(inputs["b_" + tag], dtype=np.float32)
        m = np.asarray(inputs["m_" + tag], dtype=np.float32)
        v = np.asarray(inputs["v_" + tag], dtype=np.float32)
        inv = g / np.sqrt(v + 1e-5)
        Ws[tag] = w * inv[:, None, None, None]
        Bs[tag] = be - m * inv
    init = {
        k: np.asarray(inputs[k], dtype=np.float32).reshape(-1)
        for k in ("h20", "h21", "h30", "h31")
    }

    in_maps = []
    for b in range(B):
        for orient in (0, 1):
            if orient == 0:
                img = xs[b]
                ch = slice(128, 256)
                init_a, init_b = init["h30"], init["h31"]
            else:
                img = xs[b].transpose(0, 2, 1)
                ch = slice(0, 128)
                init_a, init_b = init["h20"], init["h21"]
            # rows: top pad 1, bottom pad 2; cols: left pad 1, right pad
            # Wp-1-W (zeros; junk-read columns only hit zero weights)
            xpad = np.pad(img, ((0, 0), (1, 2), (1, Wp - 1 - W)))
            wts = np.zeros((128, 15, 128), np.float32)
            consts = np.zeros((128, 4), np.float32)
            for c, tag in enumerate(("z", "h", "s")):
                wc = Ws[tag][ch]  # (128, 64, 3, 3) [cout, cin, ky, kx]
                if orient == 1:
                    wc = wc.transpose(0, 1, 3, 2)
                for dx in range(3):
                    wts[0:64, 3 * c + dx, :] = wc[:, :, 0, dx].T
                    wts[64:128, 3 * c + dx, :] = wc[:, :, 1, dx].T
                wts[0:64, 9 + c, :] = wc[:, :, 2, 0].T
                wts[64:128, 9 + c, :] = wc[:, :, 2, 1].T
                wts[0:64, 12 + c, :] = wc[:, :, 2, 2].T
                consts[:, c] = Bs[tag][ch]
            consts[0:64, 3] = init_a
            consts[64:128, 3] = init_b
            cnp = mybir.dt.np(CONV_DT)
            if xpad.dtype != cnp:
                xpad = xpad.astype(cnp)
                wts = wts.astype(cnp)
            in_maps.append(
                {
                    "xp": np.ascontiguousarray(xpad.reshape(64, -1)),
                    "wts": wts,
                    "consts": consts,
                }
            )
    return in_maps


def gather_output(core_outs, B, H, W):
    """core_outs: list of 8 arrays (128, H*W) in core order (b-major)."""
    out = np.empty((B, 64, H, W), np.float32)
    for b in range(B):
        nat = core_outs[2 * b].astype(np.float32).reshape(2, 64, H, W)
        tr = core_outs[2 * b + 1].astype(np.float32).reshape(2, 64, W, H)
        out[b] = nat[0] + nat[1] + (tr[0] + tr[1]).transpose(0, 2, 1)
    return out


def kernel(**inputs):
    from concourse.bass_utils import run_bass_kernel_spmd

    _ensure_axon_hooks_importable()
    xs = inputs["xs"]
    B, C, H, W = xs.shape
    # the scan-init fixup ops are only needed for nonzero initial states
    # (the problem spec ships all-zero inits)
    need_fixup = any(
        np.any(np.asarray(inputs[k], dtype=np.float32))
        for k in ("h20", "h21", "h30", "h31")
    )
    nc = _get_nc(H, W, with_init_fixup=need_fixup)
    in_maps = make_in_maps(inputs, H, W)
    res = run_bass_kernel_spmd(nc, in_maps, core_ids=list(range(len(in_maps))))
    outs = [res.results[c]["out"] for c in range(len(in_maps))]
    return gather_output(outs, B, H, W)


# revision 50
# speedup vs baseline: 1.0038x; 1.0038x over previous
"""Trainium2 Bass kernel for nn_MiniGRUConv2d4 (MinGRU 4-direction conv scan).

Problem (B=4, Cin=64, Cout4=256, H=W=256):
    u_c  = conv3x3(xs, w_c) + bn_c          for c in {z, h, s}   (Cout=256)
    z    = sigmoid(u_z); hh = u_h; s = sigmoid(u_s)
    split 256 channels into 4 groups of 64; group g scans
      g=0: over H fwd, g=1: over H rev, g=2: over W fwd, g=3: over W rev
      h_i = z_i*hh_i + (1-z_i)*h_{i-1}
    out  = sum_g s_g * h_g                  (B, 64, H, W)

Sharding (8 cores): core = (batch b, orientation o).
  o=0: natural image, conv channels 128..255 (groups 2,3: W-fwd / W-rev)
  o=1: transposed image (host transposes), channels 0..127 (groups 0,1:
       H-scan becomes W-scan in the transposed frame).
Every core runs the identical program; group A (partitions 0:64) scans
forward along W, group B (partitions 64:128) scans backward (negative-
stride APs feeding the hardware scan instruction). The conv is 6 K=128
fp16 matmuls per 2-row x 256-col tile (3x3 taps: dy0/dy1 pairs packed
into the 128-partition contraction via a row-shifted second copy of the
input; dy2 rides in the upper 64 partitions with zero top-half weights).

Engine budget per core (measured): PE ~450us of matmul (the fp16 FLOP
floor is ~410us; fp8 was measured numerically and its conv error alone
breaks the 2e-2 gate, so fp16 it is), DVE ~430us (hw scan is ~2.24
ns/elem, serial in the free dim), ACT ~257us, DMA ~220us. The scan runs
UNSPLIT (channel-major) to keep DMA traffic low -- an earlier variant
row-split the scan via SBUF<->SBUF remaps to halve DVE time, but the
remap traffic saturated the 16 DMA engines (~445us busy) and starved
the PE's input loads. Instead the scan is emitted as four quarter-band
chunks interleaved between the NEXT band's b-computations, so the
in-order DVE queue never blocks the PSUM drain for more than ~2.3us.
"""

import sys
import types

import numpy as np

import concourse.bass as bass
import concourse.mybir as mybir
import concourse.tile as tile

F32 = mybir.dt.float32
AF = mybir.ActivationFunctionType
OP = mybir.AluOpType

_R = 8  # band height (output rows per band)


# ---------------------------------------------------------------------------
# Workaround: the pinned walrus rejects instructions carrying more than a
# couple of sem waits ("Too many sync wait commands", CoreV3GenImpl
# setupSyncWait). Hoist excess waits onto same-engine NOPs inserted right
# before the offending instruction.
_MAX_WAITS = 1


def _split_excess_waits(nc, max_waits=_MAX_WAITS):
    import bass_rust

    n_split = 0
    for f in nc.m.functions:
        for blk in f.blocks:
            out = []
            for inst in blk.instructions:
                si = inst.sync_info
                if si is not None and len(si.on_wait) > max_waits:
                    waits = list(si.on_wait)
                    extra, keep = waits[:-max_waits], waits[-max_waits:]
                    for i0 in range(0, len(extra), max_waits):
                        nop = mybir.InstNoOp(
                            name=f"{inst.name}_xw{i0}", ins=[], outs=[]
                        )
                        nop.engine = inst.engine
                        nop.sync_info = bass_rust.SyncInfo(
                            on_wait=extra[i0 : i0 + max_waits], on_update=[]
                        )
                        nc.register_instruction(nop)
                        out.append(nop)
                        n_split += 1
                    inst.sync_info = bass_rust.SyncInfo(
                        on_wait=keep, on_update=list(si.on_update)
                    )
                out.append(inst)
            blk.instructions = out
    return n_split


def _ensure_axon_hooks_importable():
    # bass_utils imports antenv.axon_hooks when tracing is requested; the
    # container's antenv stub lacks it. Provide a no-op registry so the
    # import never crashes (tracing then just degrades gracefully).
    try:
        import antenv.axon_hooks  # noqa: F401
    except Exception:
        try:
            import antenv

            mod = types.ModuleType("antenv.axon_hooks")
            mod._hook = None
            mod.set_axon_ntff_profile_hook = lambda h: setattr(mod, "_hook", h)
            mod.get_axon_ntff_profile_hook = lambda: mod._hook
            sys.modules["antenv.axon_hooks"] = mod
            antenv.axon_hooks = mod
        except Exception:
            pass


# ---------------------------------------------------------------------------
# Device program

# Conv operands: fp16 runs the PE at full rate (1 cyc/row) with a 10-bit
# mantissa -- conv error ~5e-4. fp8 (e4m3, 2x rate w/ DoubleRow) was
# measured at 4.6e-2 final rel err: rejected.
CONV_DT = mybir.dt.float16
CHAIN_DT = mybir.dt.float16  # z/s/a/b/h/p tiles + output (host upcasts)
# Row-split group A's scan via one SBUF->SBUF remap (halves A's serial scan
# length; B stays unsplit). Measured SLOWER (476-485us vs 453us): the remap
# saves ~66us of DVE but its SBUF<->SBUF traffic inflates PE throughput and
# adds input-load micro-stalls, and PE is the binding engine. Keep False.
SPLIT_A = False
_PADC = 5  # extra pad columns: Wp = W + 5 so every x DMA is one flat
#            descriptor per partition (row stride == Wp, junk cols are
#            zero-padded and only ever hit zero weights)


def build_nc(H, W, with_init_fixup=True):
    """One-core program; all 8 cores run it SPMD with different inputs."""
    R = _R
    RR = R + 1  # input rows resident per band (dy0/dy1 buffer)
    Wp = W + _PADC
    assert H % R == 0 and W % 2 == 0
    nbands = H // R
    cdt = CONV_DT
    wdt = CHAIN_DT
    half = R * W // 2

    nc = bass.Bass("TRN2", target_bir_lowering=False, debug=False)
    # flat padded image: row y, col k at offset y*Wp + k; rows 0..H+2
    # (top pad 1, bottom pad 2), cols: k=0 left pad, k in 1..W image,
    # k in W+1..Wp-1 right pad/junk (zeroed by host).
    xp = nc.dram_tensor("xp", [64, (H + 3) * Wp], cdt, kind="ExternalInput").ap()
    wts = nc.dram_tensor("wts", [128, 15, 128], cdt, kind="ExternalInput").ap()
    consts = nc.dram_tensor("consts", [128, 4], F32, kind="ExternalInput").ap()
    out = nc.dram_tensor("out", [128, H * W], wdt, kind="ExternalOutput").ap()

    with tile.TileContext(nc) as tc:
        with (
            tc.tile_pool(name="const", bufs=1) as cpool,
            tc.tile_pool(name="xin", bufs=4) as xpool,
            tc.tile_pool(name="work", bufs=4) as wpool,
            tc.tile_pool(name="psum", bufs=2, space="PSUM") as ppool,
        ):
            wts_sb = cpool.tile([128, 15, 128], cdt)
            nc.sync.dma_start(wts_sb[:], wts)
            cst = cpool.tile([128, 4], F32)
            nc.sync.dma_start(cst[:], consts)
            bias = [cst[:, c : c + 1] for c in range(3)]  # z, h, s
            init = cst[:, 3:4]

            def scan_chunk(k, a_p, b_p, h_p):
                # whole-group scan chunks: k=0 -> group A forward, k=1 ->
                # group B backward, each one instruction over the full band
                # (fewer DVE instruction overheads than quarter-band chunks;
                # the scan is the pacing engine's biggest block).
                if k == 0:
                    nc.vector.tensor_tensor_scan(
                        h_p[0:64, :], a_p[0:64, :], b_p[0:64, :],
                        0.0, op0=OP.mult, op1=OP.add,
                    )
                else:
                    nc.vector.tensor_tensor_scan(
                        h_p[64:128, ::-1], a_p[64:128, ::-1],
                        b_p[64:128, ::-1], 0.0,
                        op0=OP.mult, op1=OP.add,
                    )

            # prev-band state for the software-pipelined back-end
            prev = None  # (a_b, b_b, s_b, h_b, p_b, y0, absA, hsA)
            pending_out = None  # (p_b, y0) store deferred past newer loads

            def back_end_slot(k, pv):
                # DVE/Sync work for the previous band, spread across this
                # band's unit slots. SPLIT_A: k=0 -> group A split scan,
                # k=1,2 -> group B chunks (k=2 also fires the A remap-back
                # triggers, whose scan-A wait is satisfied by then so the
                # in-order Sync stream never stalls), k=3 -> idle.
                a_p, b_p, s_p, h_p, p_p, y0_p, absA_p, hsA_p = pv[:8]
                if SPLIT_A:
                    if k == 0:
                        nc.vector.tensor_tensor_scan(
                            hsA_p[:, :], absA_p[:, 0:half],
                            absA_p[:, half : 2 * half], 0.0,
                            op0=OP.mult, op1=OP.add,
                        )
                    elif k in (1, 2):
                        scan_chunk(k + 1, a_p, b_p, h_p)
                        if k == 2:
                            Rh = R // 2
                            h3 = h_p.rearrange("p (r w) -> p r w", w=W)
                            hsA3 = hsA_p.rearrange("p (r w) -> p r w", w=W)
                            nc.sync.dma_start(h3[0:64, 0:Rh, :], hsA3[0:64])
                            nc.sync.dma_start(
                                h3[0:64, Rh:R, :], hsA3[64:128]
                            )
                else:
                    lst = pv[8]
                    kk = lst[k] if k < len(lst) else None
                    if kk is not None:
                        scan_chunk(kk, a_p, b_p, h_p)

            for band in range(nbands + 1):
                if band < nbands:
                    y0 = band * R
                    # x2: dy0 rows at partitions 0:64, dy1 rows (one row
                    # down) at 64:128. One flat descriptor per partition.
                    x2f = xpool.tile([128, RR * Wp], cdt)
                    x3f = xpool.tile([128, R * Wp], cdt)
                    nc.sync.dma_start(
                        x2f[0:64], xp[:, y0 * Wp : (y0 + RR) * Wp]
                    )
                    nc.sync.dma_start(
                        x2f[64:128],
                        xp[:, (y0 + 1) * Wp : (y0 + 1 + RR) * Wp],
                    )
                    # x3: dy2 rows; lower = col+0, upper = col+1 (flat
                    # offset by one element; the tail reads the next
                    # padded row's col 0, real zero-padded memory).
                    nc.sync.dma_start(
                        x3f[0:64], xp[:, (y0 + 2) * Wp : (y0 + 2 + R) * Wp]
                    )
                    nc.sync.dma_start(
                        x3f[64:128],
                        xp[:, (y0 + 2) * Wp + 1 : (y0 + 2 + R) * Wp + 1],
                    )
                    x2 = x2f.rearrange("p (r w) -> p r w", w=Wp)
                    x3 = x3f.rearrange("p (r w) -> p r w", w=Wp)
                if pending_out is not None:
                    p_o, y0_o = pending_out
                    nc.sync.dma_start(
                        out[:, y0_o * W : (y0_o + R) * W], p_o[:]
                    )
                    pending_out = None

                if band < nbands:
                    z_b = wpool.tile([128, R * W], wdt)
                    s_b = wpool.tile([128, R * W], wdt)
                    ab = wpool.tile([128, 2 * R * W], wdt)
                    a_b = ab[:, 0 : R * W]
                    b_b = ab[:, R * W : 2 * R * W]
                    h_b = wpool.tile([128, R * W], wdt)
                    p_b = wpool.tile([128, R * W], wdt)
                    if SPLIT_A:
                        # split-layout [a|b] for group A: partition =
                        # (row-half, ch), free = [a|b] x (R/2 rows x W)
                        absA = wpool.tile([128, 2 * half], wdt)
                        hsA = wpool.tile([128, half], wdt)
                    else:
                        absA = hsA = None

                    for j0 in range(0, R, 2):  # unit: 2 output rows, N=512
                        us = []
                        for c in range(3):  # z, h, s convs
                            u = ppool.tile(
                                [128, 2 * W], F32, name=f"u{c}", tag=f"u{c}",
                                bufs=(4 if c == 1 else 2),
                            )
                            for dx in range(3):  # dy0+dy1 pairs
                                nc.tensor.matmul(
                                    u[:],
                                    wts_sb[:, 3 * c + dx, :],
                                    x2[:, j0 : j0 + 2, dx : dx + W],
                                    start=(dx == 0),
                                    stop=False,
                                )
                            # dy2 (dx0, dx1) pair
                            nc.tensor.matmul(
                                u[:],
                                wts_sb[:, 9 + c, :],
                                x3[:, j0 : j0 + 2, 0:W],
                                start=False,
                                stop=False,
                            )
                            # dy2 dx2 (upper weights zero -> K=128 uniform)
                            nc.tensor.matmul(
                                u[:],
                                wts_sb[:, 12 + c, :],
                                x3[:, j0 : j0 + 2, 2 : 2 + W],
                                start=False,
                                stop=True,
                            )
                            us.append(u)
                        sl = slice(j0 * W, (j0 + 2) * W)
                        nc.scalar.activation(
                            z_b[:, sl], us[0][:], AF.Sigmoid, bias=bias[0]
                        )
                        nc.scalar.activation(
                            s_b[:, sl], us[2][:], AF.Sigmoid, bias=bias[2]
                        )
                        # b = (u_h + bias_h) * z; reads u_h straight from
                        # PSUM (psum ports are separate from the saturated
                        # SBUF ports)
                        nc.vector.scalar_tensor_tensor(
                            b_b[:, sl], us[1][:], bias[1], z_b[:, sl],
                            op0=OP.add, op1=OP.mult,
                        )
                        # interleave one quarter of the previous band's
                        # back-end behind each unit's b
                        if prev is not None:
                            back_end_slot(j0 // 2, prev)
                else:
                    if prev is not None:
                        for k in range(4):
                            back_end_slot(k, prev)

                if prev is not None:
                    a_p, b_p, s_p, h_p, p_p, y0_p = prev[:6]
                    # (p on GpSimd measured +50us: Q7 elementwise is slow
                    # and its SBUF port locking taxes DVE. Keep on DVE.)
                    nc.vector.tensor_mul(p_p[:], s_p[:], h_p[:])
                    # the store trigger waits on p; emitting it here would
                    # gate the NEXT band's input loads on the in-order Sync
                    # stream. Defer it to the top of the next iteration
                    # (after those loads are already in flight).
                    pending_out = (p_p, y0_p)
                    prev = None

                if band < nbands:
                    # a = 1 - z (on ACT: Identity(-z + 1); DVE is scarce)
                    nc.scalar.activation(
                        a_b[:], z_b[:], AF.Identity, bias=1.0, scale=-1.0
                    )
                    a3 = a_b.rearrange("p (r w) -> p r w", w=W)
                    b3 = b_b.rearrange("p (r w) -> p r w", w=W)
                    # fold the (normally zero) scan init into b at each row
                    # edge, then zero `a` there so the flat scan restarts
                    # per row.
                    if with_init_fixup:
                        nc.vector.scalar_tensor_tensor(
                            b3[0:64, :, 0], a3[0:64, :, 0], init[0:64],
                            b3[0:64, :, 0], op0=OP.mult, op1=OP.add,
                        )
                        nc.vector.scalar_tensor_tensor(
                            b3[64:128, :, W - 1], a3[64:128, :, W - 1],
                            init[64:128], b3[64:128, :, W - 1],
                            op0=OP.mult, op1=OP.add,
                        )
                    nc.scalar.activation(
                        a3[0:64, :, 0], a3[0:64, :, 0], AF.Copy,
                        bias=0.0, scale=0.0,
                    )
                    nc.scalar.activation(
                        a3[64:128, :, W - 1], a3[64:128, :, W - 1],
                        AF.Copy, bias=0.0, scale=0.0,
                    )
                    if SPLIT_A:
                        # remap group A's a|b into the row-split layout:
                        # rows 0:R/2 -> partitions 0:64, rows R/2:R -> 64:128
                        Rh = R // 2
                        ab4 = ab.rearrange("p (pl r w) -> p pl r w", pl=2, w=W)
                        absA4 = absA.rearrange(
                            "p (pl r w) -> p pl r w", pl=2, w=W
                        )
                        nc.sync.dma_start(absA4[0:64], ab4[0:64, :, 0:Rh, :])
                        nc.sync.dma_start(
                            absA4[64:128], ab4[0:64, :, Rh:R, :]
                        )
                    prev = (
                        a_b, b_b, s_b, h_b, p_b, y0, absA, hsA,
                        # A after unit 0's b, B after unit 2's b: spreads the
                        # two 4.6us scan blocks so psum drains stay in slack
                        [0, None, 1, None],
                    )
            if pending_out is not None:
                p_o, y0_o = pending_out
                nc.sync.dma_start(out[:, y0_o * W : (y0_o + R) * W], p_o[:])
    _split_excess_waits(nc)
    return nc


# ---------------------------------------------------------------------------
# Host side

_NC_CACHE = {}


def _get_nc(H, W, with_init_fixup=True):
    key = (H, W, with_init_fixup)
    if key not in _NC_CACHE:
        _NC_CACHE[key] = build_nc(H, W, with_init_fixup)
    return _NC_CACHE[key]


def make_in_maps(inputs, H, W):
    """Build the 8 per-core input dicts from the full problem inputs."""
    xs = np.ascontiguousarray(np.asarray(inputs["xs"], dtype=np.float32))
    B = xs.shape[0]
    Wp = W + _PADC
    Ws, Bs = {}, {}
    for tag in ("z", "h", "s"):
        w = np.asarray(inputs["w_" + tag], dtype=np.float32)
        g = np.asarray(inputs["g_" + tag], dtype=np.float32)
        be = np.asarray(inputs["b_" + tag], dtype=np.float32)
        m = np.asarray(inputs["m_" + tag], dtype=np.float32)
        v = np.asarray(inputs["v_" + tag], dtype=np.float32)
        inv = g / np.sqrt(v + 1e-5)
        Ws[tag] = w * inv[:, None, None, None]
        Bs[tag] = be - m * inv
    init = {
        k: np.asarray(inputs[k], dtype=np.float32).reshape(-1)
        for k in ("h20", "h21", "h30", "h31")
    }

    in_maps = []
    for b in range(B):
        for orient in (0, 1):
            if orient == 0:
                img = xs[b]
                ch = slice(128, 256)
                init_a, init_b = init["h30"], init["h31"]
            else:
                img = xs[b].transpose(0, 2, 1)
                ch = slice(0, 128)
                init_a, init_b = init["h20"], init["h21"]
            # rows: top pad 1, bottom pad 2; cols: left pad 1, right pad
            # Wp-1-W (zeros; junk-read columns only hit zero weights)
            xpad = np.pad(img, ((0, 0), (1, 2), (1, Wp - 1 - W)))
            wts = np.zeros((128, 15, 128), np.float32)
            consts = np.zeros((128, 4), np.float32)
            for c, tag in enumerate(("z", "h", "s")):
                wc = Ws[tag][ch]  # (128, 64, 3, 3) [cout, cin, ky, kx]
                if orient == 1:
                    wc = wc.transpose(0, 1, 3, 2)
                for dx in range(3):
                    wts[0:64, 3 * c + dx, :] = wc[:, :, 0, dx].T
                    wts[64:128, 3 * c + dx, :] = wc[:, :, 1, dx].T
                wts[0:64, 9 + c, :] = wc[:, :, 2, 0].T
                wts[64:128, 9 + c, :] = wc[:, :, 2, 1].T
                wts[0:64, 12 + c, :] = wc[:, :, 2, 2].T
                consts[:, c] = Bs[tag][ch]
            consts[0:64, 3] = init_a
            consts[64:128, 3] = init_b
            cnp = mybir.dt.np(CONV_DT)
            if xpad.dtype != cnp:
                xpad = xpad.astype(cnp)
                wts = wts.astype(cnp)
            in_maps.append(
                {
                    "xp": np.ascontiguousarray(xpad.reshape(64, -1)),
                    "wts": wts,
                    "consts": consts,
                }
            )
    return in_maps


def gather_output(core_outs, B, H, W):
    """core_outs: list of 8 arrays (128, H*W) in core order (b-major)."""
    out = np.empty((B, 64, H, W), np.float32)
    for b in range(B):
        nat = core_outs[2 * b].astype(np.float32).reshape(2, 64, H, W)
        tr = core_outs[2 * b + 1].astype(np.float32).reshape(2, 64, W, H)
        out[b] = nat[0] + nat[1] + (tr[0] + tr[1]).transpose(0, 2, 1)
    return out


def kernel(**inputs):
    from concourse.bass_utils import run_bass_kernel_spmd

    _ensure_axon_hooks_importable()
    xs = inputs["xs"]
    B, C, H, W = xs.shape
    # the scan-init fixup ops are only needed for nonzero initial states
    # (the problem spec ships all-zero inits)
    need_fixup = any(
        np.any(np.asarray(inputs[k], dtype=np.float32))
        for k in ("h20", "h21", "h30", "h31")
    )
    nc = _get_nc(H, W, with_init_fixup=need_fixup)
    in_maps = make_in_maps(inputs, H, W)
    res = run_bass_kernel_spmd(nc, in_maps, core_ids=list(range(len(in_maps))))
    outs = [res.results[c]["out"] for c in range(len(in_maps))]
    return gather_output(outs, B, H, W)
